# revision 11
# baseline (speedup 1.0000x reference)
"""Trainium2 Bass kernel for nn_AttnProcessor_LoRA_Capture (cross-attention
with LoRA on K/V/out projections + subject-token score normalization).

Strategy: pure data-parallel over batch (B=8 across 8 NeuronCores, no
collectives). Per core (one batch element, b):
  - LoRA deltas are folded into the K/V/out weights on the host (exact).
  - Q projection runs in fp8(e4m3) with DoubleRow perf mode (2 k-planes per
    matmul, K=256 per instruction). The 1/sqrt(HD) score scale and the fp8
    weight pre-scale are compensated in the softmax exp scale (host-side).
  - The subject-token normalization is linear: the per-(s,h) mean of scores
    over queries only needs qbar = mean_q(hs) @ Wq.T, so the bias factor
    g[s,h] = exp(-csf * mean_score) is computed ON HOST and folded into the
    AV stationary operand (v * g) and the softmax-denominator weights.
    exp(logit + bias) = g * exp(logit), so the device exp needs no bias.
  - Score matmuls for a head pair (K=64 each) run concurrently in separate
    PE row-groups (tile_position rows 0-63 / 64-127), writing one 2-bank
    PSUM tile; a single [77,1024] exp covers both heads.
  - Softmax denominators come from a col-tiled concurrent pair of
    g-weighted ones-matmuls (M=64 each into disjoint PSUM partition halves);
    AV for the pair is col-tiled the same way (as in the baseline).
  - Out projection drains through the scalar engine (plain copy, fp32);
    the output bias bo is added on host.
All big matmul operands are fp8/bf16 (fp32 PSUM accumulation); softmax
statistics stay fp32.
"""

import numpy as np

B, LQ, S, D = 8, 4096, 77, 1024
H, HD, R = 16, 64, 192
LORA_SCALE = 16.0 / 192.0
NCORES = 8
P = 128
QC = 512            # query chunk (free dim of score/AV matmuls)
NCH = LQ // QC      # 8 chunks
ET = D // P         # 8 contraction tiles over D
DT = D // P         # 8 d-tiles (= pairs of heads)
SCALE = 1.0 / 8.0   # 1/sqrt(HD)
WQ_FP8_SCALE = 16.0                     # keeps fp8 Wq values in normal range
SCORE_DESCALE = SCALE / WQ_FP8_SCALE    # device scores are 1/SCORE_DESCALE x true

_CACHED_NC = None


def _build_nc():
    import concourse.mybir as mybir
    import concourse.tile as tile
    from concourse import bacc

    f32 = mybir.dt.float32
    bf16 = mybir.dt.bfloat16
    fp8 = mybir.dt.float8e4
    Exp = mybir.ActivationFunctionType.Exp
    Copy = mybir.ActivationFunctionType.Copy
    mult = mybir.AluOpType.mult
    DR = mybir.MatmulPerfMode.DoubleRow

    nc = bacc.Bacc(None, target_bir_lowering=False)

    hsT_d = nc.dram_tensor("hsT", [D, LQ], fp8, kind="ExternalInput")
    ehsT_d = nc.dram_tensor("ehsT", [D, S], bf16, kind="ExternalInput")
    wqT_d = nc.dram_tensor("WqT", [D, D], fp8, kind="ExternalInput")
    wkT_d = nc.dram_tensor("WkT", [D, D], bf16, kind="ExternalInput")
    wvT_d = nc.dram_tensor("WvT", [D, D], bf16, kind="ExternalInput")
    woT_d = nc.dram_tensor("WoT", [D, D], bf16, kind="ExternalInput")
    alpha_d = nc.dram_tensor("alpha", [S, 1], f32, kind="ExternalInput")
    gv_d = nc.dram_tensor("gv", [S, H], f32, kind="ExternalInput")
    maskg_d = nc.dram_tensor("maskg", [S, DT * P], bf16, kind="ExternalInput")
    out_d = nc.dram_tensor("out", [LQ, D], bf16, kind="ExternalOutput")

    with tile.TileContext(nc) as tc:
        with (
            tc.tile_pool(name="const", bufs=1) as const,
            tc.tile_pool(name="wq", bufs=1) as wqp,
            tc.tile_pool(name="wo", bufs=1) as wop,
            tc.tile_pool(name="qt", bufs=1) as qtp,
            tc.tile_pool(name="hs", bufs=3) as hsp,
            tc.tile_pool(name="ot", bufs=2) as otp,
            tc.tile_pool(name="ep", bufs=4) as epool,
            tc.tile_pool(name="rc", bufs=2) as rcp,
            tc.tile_pool(name="fin", bufs=3) as finp,
            tc.tile_pool(name="small", bufs=1) as smallp,
        ):
            # ---------------- constant / weight DMAs ----------------
            # KV-phase inputs stream first (the KV matmuls run first and
            # warm the PE while wq/hs still arrive), wq/hs interleave after.
            # wk is split into per-etile tiles so the first matmul only
            # waits on ehsT + one 256 KB slice.
            ehsT_t = const.tile([P, ET, S], bf16, tag="ehsTall", name="ehsTall")
            nc.sync.dma_start(ehsT_t, ehsT_d.rearrange("(eo p) s -> p eo s", p=P))
            ehsT_sb = [ehsT_t[:, e, :] for e in range(ET)]
            wq_t = wqp.tile([P, ET, D], fp8, tag="wqall", name="wqall")
            wqT_r = wqT_d.rearrange("(eo p) d -> p eo d", p=P)
            hsT_r = hsT_d.rearrange("(eo p) q -> p eo q", p=P)
            hs_pre = [hsp.tile([P, ET, QC], fp8, tag="hs", name="hs")
                      for _ in range(2)]

            kT_sb = [const.tile([P, S], bf16, tag=f"kT{p}", name=f"kT{p}")
                     for p in range(DT)]
            vg_sb = const.tile([S, D], bf16, tag="vg", name="vg")
            qt_sb = [qtp.tile([P, LQ], bf16, tag=f"qt{d}", name=f"qt{d}")
                     for d in range(DT)]

            with tc.tile_pool(name="pA", bufs=4, space="PSUM") as pA:
                # ======== KV phase (first: dense bf16 stream warms HAM) ====
                with tc.tile_pool(name="wkv", bufs=1) as kvp:
                    wkT_r = wkT_d.rearrange("(eo p) d -> p eo d", p=P)
                    wk_sb = []
                    for e in range(ET):
                        wk_e = kvp.tile([P, D], bf16, tag=f"wk{e}",
                                        name=f"wk{e}")
                        nc.sync.dma_start(wk_e, wkT_r[:, e, :])
                        wk_sb.append(wk_e)
                    wvT_r = wvT_d.rearrange("(eo p) d -> p eo d", p=P)
                    wv_sb = []
                    for e in range(ET):
                        wv_e = kvp.tile([P, D], bf16, tag=f"wv{e}",
                                        name=f"wv{e}")
                        nc.sync.dma_start(wv_e, wvT_r[:, e, :])
                        wv_sb.append(wv_e)
                    # wq/hs issue from the (otherwise idle) gpsimd queue and
                    # wo/small tiles from scalar, so ~45 prologue DMAs don't
                    # serialize on the single sync queue (~0.65us issue each)
                    alpha_sb = smallp.tile([S, 1], f32, tag="alpha", name="alpha")
                    nc.scalar.dma_start(alpha_sb, alpha_d[:, :])
                    gv_sb = smallp.tile([S, H], f32, tag="gv", name="gv")
                    nc.scalar.dma_start(gv_sb, gv_d[:, :])
                    maskg_sb = smallp.tile([S, DT * P], bf16, tag="maskg",
                                           name="maskg")
                    nc.scalar.dma_start(maskg_sb, maskg_d[:, :])
                    for e in range(ET):
                        nc.gpsimd.dma_start(wq_t[:, e, :], wqT_r[:, e, :])
                        nc.gpsimd.dma_start(hs_pre[0][:, e, :],
                                            hsT_r[:, e, 0:QC])
                    for e in range(ET):
                        nc.gpsimd.dma_start(hs_pre[1][:, e, :],
                                            hsT_r[:, e, QC:2 * QC])
                    wo_t = wop.tile([P, ET, D], bf16, tag="woall", name="woall")
                    nc.scalar.dma_start(wo_t, woT_d.rearrange("(eo p) d -> p eo d", p=P))

                    # ---- kT[d, s] per pair-tile ----
                    for p in range(DT):
                        ps = pA.tile([P, QC], f32, tag="mm", name="mm")[:, :S]
                        for e in range(ET):
                            nc.tensor.matmul(ps, lhsT=wk_sb[e][:, p * P:(p + 1) * P],
                                             rhs=ehsT_sb[e],
                                             start=(e == 0), stop=(e == ET - 1))
                        nc.vector.tensor_copy(kT_sb[p], ps)

                    # ---- V[s, d], scaled per head by g during the drain ----
                    for dc in range(2):
                        ps = pA.tile([P, QC], f32, tag="mm", name="mm")[:S, :]
                        for e in range(ET):
                            nc.tensor.matmul(ps, lhsT=ehsT_sb[e],
                                             rhs=wv_sb[e][:, dc * QC:(dc + 1) * QC],
                                             start=(e == 0), stop=(e == ET - 1))
                        for hh in range(8):
                            h = 8 * dc + hh
                            nc.vector.tensor_scalar_mul(
                                vg_sb[:, dc * QC + hh * HD:dc * QC + (hh + 1) * HD],
                                ps[:, hh * HD:(hh + 1) * HD],
                                gv_sb[:, h:h + 1])

                # ======== phase A: Q projection (fp8 DoubleRow) ============
                # chunks 0..6 here; chunk 7's projection is deferred into
                # phase C's first chunk to keep the PE dense there
                def emit_qproj(c, d, hs_t, pool, drain_dve):
                    ps = pool.tile([P, QC], f32, tag=pool._qtag, name=pool._qtag)
                    for j in range(ET // 2):
                        nc.tensor.matmul(
                            ps,
                            lhsT=wq_t[:, 2 * j:2 * j + 2, d * P:(d + 1) * P],
                            rhs=hs_t[:, 2 * j:2 * j + 2, :],
                            start=(j == 0), stop=(j == ET // 2 - 1),
                            perf_mode=DR)
                    tgt = qt_sb[d][:, c * QC:(c + 1) * QC]
                    if drain_dve:
                        nc.vector.tensor_copy(tgt, ps)
                    else:
                        nc.scalar.activation(tgt, ps, Copy)

                pA._qtag = "mm"
                for c in range(NCH - 1):
                    if c < 2:
                        hs_t = hs_pre[c]
                    else:
                        hs_t = hsp.tile([P, ET, QC], fp8, tag="hs", name="hs")
                        nc.gpsimd.dma_start(hs_t, hsT_r[:, :, c * QC:(c + 1) * QC])
                    for d in range(DT):
                        emit_qproj(c, d, hs_t, pA, drain_dve=(d % 2 == 1))
                hs_last = hsp.tile([P, ET, QC], fp8, tag="hs", name="hs")
                nc.gpsimd.dma_start(hs_last,
                                    hsT_r[:, :, (NCH - 1) * QC:NCH * QC])
            wo_sb = [wo_t[:, e, :] for e in range(ET)]

            # ============ phase C: scores/softmax/AV/out-proj ============
            # Out-proj is software-pipelined by one chunk: its matmul groups
            # are interleaved between attention pairs so the PE has dense
            # work while the scalar engine runs the exps.  Emission order per
            # pair p is scores(p) -> op-group(p, c-1) -> rs/av(p-1): by the
            # time the in-order PE queue reaches rs/av(p-1), exp(p-1) has
            # completed, so the queue never head-of-line blocks.
            with (
                tc.tile_pool(name="psc", bufs=2, space="PSUM") as psc,
                tc.tile_pool(name="prs", bufs=1, space="PSUM") as prs,
                tc.tile_pool(name="pav", bufs=1, space="PSUM") as pav,
                tc.tile_pool(name="pout", bufs=2, space="PSUM") as pout,
            ):
                def emit_scores(c, p):
                    # score pair: concurrent row-group matmuls into one
                    # 2-bank PSUM tile; one exp covers both heads (bias is
                    # folded into vg/maskg)
                    ps2 = psc.tile([P, 2 * QC], f32, tag="score", name="score")
                    nc.tensor.matmul(
                        ps2[:S, 0:QC],
                        lhsT=kT_sb[p][0:HD, :],
                        rhs=qt_sb[p][0:HD, c * QC:(c + 1) * QC],
                        start=True, stop=True)
                    nc.tensor.matmul(
                        ps2[:S, QC:2 * QC],
                        lhsT=kT_sb[p][HD:P, :],
                        rhs=qt_sb[p][HD:P, c * QC:(c + 1) * QC],
                        start=True, stop=True)
                    e_t = epool.tile([S, 2 * QC], bf16, tag="E", name="E")
                    nc.scalar.activation(e_t, ps2[:S, :], Exp, scale=alpha_sb)
                    return e_t

                def emit_rsav(p, e_t, otc):
                    # denominators: col-tiled concurrent pair (M=64 each)
                    ps_rs = prs.tile([P, QC], f32, tag="rs", name="rs")
                    nc.tensor.matmul(ps_rs[0:HD, :],
                                     lhsT=maskg_sb[:, p * P:p * P + HD],
                                     rhs=e_t[:, 0:QC], start=True, stop=True,
                                     tile_position=(0, 0))
                    nc.tensor.matmul(ps_rs[HD:P, :],
                                     lhsT=maskg_sb[:, p * P + HD:(p + 1) * P],
                                     rhs=e_t[:, QC:2 * QC], start=True,
                                     stop=True, tile_position=(0, HD))
                    recip = rcp.tile([P, QC], f32, tag="recip", name="recip")
                    nc.vector.reciprocal_approx_fast(recip, ps_rs)
                    # AV for the head pair, col-tiled into one PSUM tile
                    ps_av = pav.tile([P, QC], f32, tag="av", name="av")
                    nc.tensor.matmul(ps_av[0:HD, :],
                                     lhsT=vg_sb[:, (2 * p) * HD:(2 * p + 1) * HD],
                                     rhs=e_t[:, 0:QC], start=True, stop=True,
                                     tile_position=(0, 0))
                    nc.tensor.matmul(ps_av[HD:P, :],
                                     lhsT=vg_sb[:, (2 * p + 1) * HD:(2 * p + 2) * HD],
                                     rhs=e_t[:, QC:2 * QC], start=True,
                                     stop=True, tile_position=(0, HD))
                    nc.vector.tensor_tensor(otc[p], ps_av, recip, mult)

                def emit_opgroup(c, g, otc, drain_dve=True):
                    # out projection group g of chunk c (bias added on host)
                    qs, ec = g // 2, g % 2
                    ps_o = pout.tile([P, QC], f32, tag="out", name="out")
                    for p in range(DT):
                        nc.tensor.matmul(
                            ps_o, lhsT=otc[p][:, qs * P:(qs + 1) * P],
                            rhs=wo_sb[p][:, ec * QC:(ec + 1) * QC],
                            start=(p == 0), stop=(p == DT - 1))
                    # drain on the vector engine (scalar stays free for the
                    # exps — an ACT-queue copy here head-of-line blocks them)
                    fin = finp.tile([P, QC], bf16, tag="fin", name="fin")
                    if drain_dve:
                        nc.vector.tensor_copy(fin, ps_o)
                    else:
                        nc.scalar.activation(fin, ps_o, Copy)
                    nc.sync.dma_start(
                        out_d[c * QC + qs * P:c * QC + (qs + 1) * P,
                              ec * QC:(ec + 1) * QC], fin)

                pout._qtag = "out"
                otc_prev = None
                for c in range(NCH):
                    otc = [otp.tile([P, QC], bf16, tag=f"ot{p}", name=f"ot{p}")
                           for p in range(DT)]
                    es_tiles = [None] * DT
                    for p in range(DT + 1):
                        if p < DT:
                            es_tiles[p] = emit_scores(c, p)
                            if otc_prev is not None:
                                emit_opgroup(c - 1, p, otc_prev)
                            else:
                                # chunk 0: deferred chunk-7 Q projection keeps
                                # the PE dense (pout banks are free here)
                                emit_qproj(NCH - 1, p, hs_last, pout,
                                           drain_dve=(p % 2 == 1))
                        if p >= 1:
                            emit_rsav(p - 1, es_tiles[p - 1], otc)
                    otc_prev = otc
                # tail: last chunk's out-proj; alternate drain engines (the
                # exps are done, so the scalar engine is free to help)
                for g in range(DT):
                    emit_opgroup(NCH - 1, g, otc_prev, drain_dve=(g % 2 == 1))
    nc.compile()
    return nc


def get_nc():
    global _CACHED_NC
    if _CACHED_NC is None:
        _CACHED_NC = _build_nc()
    return _CACHED_NC


def make_in_maps(inputs):
    import ml_dtypes
    bf16 = ml_dtypes.bfloat16
    fp8 = ml_dtypes.float8_e4m3

    hs = np.asarray(inputs["hidden_states"], np.float32)
    ehs = np.asarray(inputs["encoder_hidden_states"], np.float32)
    Wq = np.asarray(inputs["Wq"], np.float32)
    Wk = np.asarray(inputs["Wk"], np.float32)
    Wv = np.asarray(inputs["Wv"], np.float32)
    Wo = np.asarray(inputs["Wo"], np.float32)
    Ak = np.asarray(inputs["Ak"], np.float32)
    Bk = np.asarray(inputs["Bk"], np.float32)
    Av = np.asarray(inputs["Av"], np.float32)
    Bv = np.asarray(inputs["Bv"], np.float32)
    Ao = np.asarray(inputs["Ao"], np.float32)
    Bo = np.asarray(inputs["Bo"], np.float32)
    csf = float(np.asarray(inputs["cross_attn_scale_factor"]))
    subj_b = np.asarray(inputs["subj_b"]).astype(np.int64)
    subj_n = np.asarray(inputs["subj_n"]).astype(np.int64)

    def cvt(a):
        return np.ascontiguousarray(a).astype(bf16)

    # Fold LoRA deltas into the base weights (exact):
    #   x @ W.T + s*(x @ A.T) @ B.T = x @ (W + s*B@A).T
    Wk_eff = Wk + LORA_SCALE * (Bk @ Ak)
    Wv_eff = Wv + LORA_SCALE * (Bv @ Av)
    Wo_eff = Wo + LORA_SCALE * (Bo @ Ao)

    WqT8 = np.ascontiguousarray(Wq.T * WQ_FP8_SCALE).astype(fp8)
    WkT = cvt(Wk_eff.T)
    WvT = cvt(Wv_eff.T)
    WoT = cvt(Wo_eff.T)
    shared = dict(WqT=WqT8, WkT=WkT, WvT=WvT, WoT=WoT)

    in_maps = []
    for b in range(NCORES):
        mask = np.zeros(S, bool)
        mask[subj_n[subj_b == b]] = True
        # device scores are scaled by 1/SCORE_DESCALE; compensate in exp scale
        alpha = (np.where(mask, csf, 1.0) * SCORE_DESCALE).astype(np.float32)
        # subject normalization bias, computed host-side (linear in scores):
        #   mean_q score[s,h,q] = SCALE * k[s,h,:] . qbar_h,
        #   qbar = mean_q(hs) @ Wq.T
        qbar = hs[b].mean(axis=0) @ Wq.T                      # [D]
        k_host = ehs[b] @ Wk_eff.T                            # [S, D]
        mu = np.einsum('shd,hd->sh', k_host.reshape(S, H, HD),
                       qbar.reshape(H, HD)) * SCALE           # [S, H]
        g = np.where(mask[:, None], np.exp(-csf * mu), 1.0).astype(np.float32)
        maskg = np.repeat(g, HD, axis=1)                      # [S, H*HD]
        m = dict(shared)
        m["hsT"] = np.ascontiguousarray(hs[b].T).astype(fp8)
        m["ehsT"] = cvt(ehs[b].T)
        m["alpha"] = alpha.reshape(S, 1)
        m["gv"] = g
        m["maskg"] = maskg.astype(bf16)
        in_maps.append(m)
    return in_maps


def _install_profile_hook():
    """Make trace=True work in this container: provide the antenv.axon_hooks
    registry that concourse expects and register the ctypes NTFF hook."""
    import sys
    import types
    if "antenv.axon_hooks" not in sys.modules:
        mod = types.ModuleType("antenv.axon_hooks")
        mod._hook = None

        def set_axon_ntff_profile_hook(h, _mod=mod):
            _mod._hook = h

        def get_axon_ntff_profile_hook(_mod=mod):
            return _mod._hook

        mod.set_axon_ntff_profile_hook = set_axon_ntff_profile_hook
        mod.get_axon_ntff_profile_hook = get_axon_ntff_profile_hook
        sys.modules["antenv.axon_hooks"] = mod
        try:
            import antenv
            antenv.axon_hooks = mod
        except ImportError:
            pass
    mod = sys.modules["antenv.axon_hooks"]
    if mod.get_axon_ntff_profile_hook() is None:
        try:
            from trn_agent_boot.trn_boot import _ntff_profile_via_ctypes
            hook = _ntff_profile_via_ctypes("/opt/axon/libaxon_pjrt.so")
            if hook is not None:
                mod.set_axon_ntff_profile_hook(hook)
        except Exception as e:  # degrade to no tracing
            print(f"profile hook install failed: {e}")


def run(inputs, trace=False):
    from concourse.bass_utils import run_bass_kernel_spmd
    if trace:
        _install_profile_hook()
    nc = get_nc()
    in_maps = make_in_maps(inputs)
    res = run_bass_kernel_spmd(nc, in_maps, core_ids=list(range(NCORES)),
                               trace=trace)
    bo = np.asarray(inputs["bo"], np.float32)
    out = np.stack([np.asarray(res.results[i]["out"]).astype(np.float32)
                    for i in range(NCORES)]) + bo[None, None, :]
    return out, res


def kernel(**inputs):
    out, _ = run(inputs, trace=False)
    return out
